# revision 6
# baseline (speedup 1.0000x reference)
"""Trainium2 Bass kernel for nn_LAtAttrRobertaSelfAttention.

ref:  q = split_heads(x @ Wq.T + bq); k, v likewise
      scores = q k^T / sqrt(D) + attention_mask          [B,H,S,S]
      probs  = softmax(scores, -1) * link_mask           (link broadcast over H)
      out    = merge_heads(probs @ v)                    [B,S,DM]

Sharding: 8 cores = 4 batches x 2 head-groups (8 heads each).

Per-core layout (all matmul operands bf16, fp32 accumulate):
  xT   [DM,S]   hidden_states[b].T       (host pre-transposed)
  wq/wk/wv [DM,OC] weight-slice.T for this head group (OC=512; q pre-scaled
      by 1/sqrt(D) on the host)
  qT,kT [OC,S]  = W.T-stationary matmuls; v [S,OC] = xT-stationary.
  sT[ki,qi] = kT-slice stationary @ qT-slice moving (softmax axis on
      partitions: probs are contraction-major for ctx, no transposes).

v2 over the serial baseline — exploit PE tile_position concurrency
(HW-measured: row-paired K=64 and col-paired M=64 matmuls with different
moving operands both run 2x):
  * scores PSUM is a manual 3-slice ring [128, 3*1024] (6 banks). The 4
    matmuls of a (c, head-pair) group write TWO slices with row-group
    interleaved order (h0q0, h1q0, h0q1, h1q1) -> 2 concurrent slots.
    h0 always lands in the lower-addressed slice so downstream tiles have
    a uniform (h0|h1) layout.
  * one ACT exp per slice-PAIR (FD=2048, ~12% cheaper per element than
    FD=1024 and frees two slices at once -- which is what makes the next
    4-matmul group pairable).
  * ctx and Z all-ones chains emit col-group alternating (tile_position
    (0,0)/(0,64)) -> 2x.
  * Z via DVE pair-sums of ex2 tiles (FD=2048 adds), denominator
    broadcast by all-ones matmuls, 1/Z = exp(-ln Z) on ACT (one table set).
  * deferred qk/v projection matmuls (serial K=128, no tiling possible)
    fill the PE between exp-paced score groups, as in the baseline.
  Output written as outT [OC,S] bf16; host transposes back.
"""

import functools

import numpy as np
import ml_dtypes

BF16 = ml_dtypes.bfloat16

B, S, DM, H = 4, 1024, 1024, 16
D = 64                # head dim
HG = 2                # head groups (tensor-parallel factor)
HL = H // HG          # 8 heads per core
OC = HL * D           # 512 output channels per core
NCORES = B * HG       # 8
KC = DM // 128        # 8 contraction chunks of 128
MC = OC // 128        # 4 o-chunks
QHALF = 512           # qi processed in halves (PSUM bank = 512 fp32)
RING = 3              # scores psum ring slices of [128, 1024]


def _patch_tile_drain():
    """walrus in this container rejects instructions carrying more than one
    sync wait ("Too many sync wait commands"). Tile freely attaches several.
    Two patches: (1) split excess waits off every scheduled instruction onto
    single-wait NoOps committed just before it (same engine, so program order
    preserves the blocking semantics); (2) same treatment for the TileContext
    exit drain, which carries one wait per live proc."""
    import concourse.mybir as mybir
    import concourse.tile as ctile
    from concourse.vector_clock import ScopedClock

    MAXW = 1

    if not getattr(ctile.TileContext, "_ant_split_waits_patched", False):
        orig_commit = ctile.TileContext._commit_instruction

        def _commit_instruction(self, inst, lazy_reg_writes=True):
            if isinstance(inst, mybir.Instruction):
                si = inst.sync_info
                waits = list(si.on_wait) if si is not None and si.on_wait else []
                if len(waits) > MAXW:
                    for i in range(0, len(waits) - MAXW, MAXW):
                        nop = mybir.InstNoOp(
                            name=f"{inst.name}_w{i}",
                            engine=inst.engine,
                            sync_info=mybir.SyncInfo(
                                on_wait=waits[i:i + MAXW], on_update=[]),
                            bass_nofuse=True,
                        )
                        orig_commit(self, nop, lazy_reg_writes)
                    inst.sync_info = mybir.SyncInfo(
                        on_wait=waits[len(waits) - MAXW:],
                        on_update=(si.on_update or []),
                    )
            return orig_commit(self, inst, lazy_reg_writes)

        ctile.TileContext._commit_instruction = _commit_instruction
        ctile.TileContext._ant_split_waits_patched = True

    def _drain_and_barrier(self, tick_clock, wait_clock):
        nc = self.nc
        drain_inst = nc.sync.drain()
        wait_clock.add_sem_waits(
            drain_inst.ins, ScopedClock({None: tick_clock.global_clock})
        )
        si = drain_inst.ins.sync_info
        waits = list(si.on_wait or []) if si is not None else []
        if len(waits) > 1:
            drain_inst.ins.sync_info = mybir.SyncInfo(
                on_wait=[waits[0]], on_update=(si.on_update or [])
            )
            for w in waits[1:]:
                extra = nc.sync.drain()
                extra.ins.sync_info = mybir.SyncInfo(on_wait=[w], on_update=[])
        nc.all_engine_barrier()
        assert self.sems is not None
        popped = nc._tile_sem_poison_stack.pop()
        assert popped is self._sem_poison
        nc.clear_and_free_semaphores(list(self.sems.allocated().values()))
        nc.all_engine_barrier()

    ctile.TileContext._drain_and_barrier = _drain_and_barrier


@functools.lru_cache(maxsize=None)
def _build(apply_qkbias: bool, apply_vbias: bool, apply_am: bool):
    import concourse.bass as bass
    import concourse.mybir as mybir
    import concourse.tile as tile

    _patch_tile_drain()

    f32 = mybir.dt.float32
    bf16 = mybir.dt.bfloat16
    AF = mybir.ActivationFunctionType

    nc = bass.Bass("TRN2")
    xT = nc.dram_tensor("xT", [DM, S], bf16, kind="ExternalInput")
    wq = nc.dram_tensor("wq", [DM, OC], bf16, kind="ExternalInput")
    wk = nc.dram_tensor("wk", [DM, OC], bf16, kind="ExternalInput")
    wv = nc.dram_tensor("wv", [DM, OC], bf16, kind="ExternalInput")
    lkT = nc.dram_tensor("lkT", [S, S], bf16, kind="ExternalInput")
    am = nc.dram_tensor("am", [128, KC], f32, kind="ExternalInput")
    bqs = nc.dram_tensor("bqs", [128, MC], f32, kind="ExternalInput")
    bks = nc.dram_tensor("bks", [128, MC], f32, kind="ExternalInput")
    bvb = nc.dram_tensor("bvb", [1, OC], bf16, kind="ExternalInput")
    outT = nc.dram_tensor("outT", [OC, S], bf16, kind="ExternalOutput")

    with tile.TileContext(nc) as tc:
        with (
            tc.tile_pool(name="consts", bufs=1) as consts,
            tc.tile_pool(name="qkv", bufs=1) as qkvp,
            tc.tile_pool(name="expp", bufs=4) as expp,
            tc.tile_pool(name="ptp", bufs=14) as ptp,
            tc.tile_pool(name="parp", bufs=6) as parp,
            tc.tile_pool(name="rbp", bufs=2) as rbp,
            tc.tile_pool(name="lnp", bufs=2) as lnp,
            tc.tile_pool(name="outp", bufs=2) as outp,
            tc.tile_pool(name="psr", bufs=1, space="PSUM") as psr,
            tc.tile_pool(name="psc", bufs=1, space="PSUM") as psc,
            tc.tile_pool(name="psz", bufs=1, space="PSUM") as psz,
        ):
            # ---- constant loads (interleaved so qk0 matmuls start ASAP) ------
            x_sb = [consts.tile([128, S], bf16, name=f"x{k}", tag=f"x{k}")
                    for k in range(KC)]
            w_sb = {wname: [consts.tile([128, OC], bf16, name=f"w{wname}{k}",
                                        tag=f"w{wname}{k}")
                            for k in range(KC)]
                    for wname in ("q", "k", "v")}
            lk_sb = [consts.tile([128, S], bf16, name=f"lk{c}", tag=f"lk{c}")
                     for c in range(KC)]
            # tiny per-partition constants FIRST — the exp stream's bias tile
            # (am_sb) must not queue behind 7MB of bulk input
            am_sb = consts.tile([128, KC], f32, name="am_sb", tag="am_sb")
            nc.sync.dma_start(out=am_sb, in_=am[:, :])
            bqs_sb = consts.tile([128, MC], f32, name="bqs_sb", tag="bqs_sb")
            nc.sync.dma_start(out=bqs_sb, in_=bqs[:, :])
            bks_sb = consts.tile([128, MC], f32, name="bks_sb", tag="bks_sb")
            nc.sync.dma_start(out=bks_sb, in_=bks[:, :])
            if apply_vbias:
                bvb_sb = consts.tile([128, OC], bf16, name="bvb_sb", tag="bvb_sb")
                nc.sync.dma_start(out=bvb_sb,
                                  in_=bvb[0:1, :].partition_broadcast(128))

            # input DMA is HBM-bandwidth-bound on one queue; order chunks by
            # when the compute needs them (x+wq first for qk0/qk1)
            for k in range(KC):
                nc.sync.dma_start(out=x_sb[k], in_=xT[k * 128:(k + 1) * 128, :])
                nc.sync.dma_start(out=w_sb["q"][k],
                                  in_=wq[k * 128:(k + 1) * 128, :])
            for k in range(KC):
                nc.sync.dma_start(out=w_sb["k"][k],
                                  in_=wk[k * 128:(k + 1) * 128, :])
            for k in range(KC):
                nc.sync.dma_start(out=w_sb["v"][k],
                                  in_=wv[k * 128:(k + 1) * 128, :])
            for c in range(KC):
                nc.sync.dma_start(out=lk_sb[c],
                                  in_=lkT[c * 128:(c + 1) * 128, :])
            ones_sb = consts.tile([128, 64], bf16, name="ones_sb", tag="ones_sb")
            nc.vector.memset(ones_sb, 1.0)
            # dummy matmuls on scratch data while the input DMA streams: the
            # PE's HAM clock-gate needs ~3.4us of sustained activity to lift
            # the array from 1.2GHz to 2.4GHz, so warm it up BEFORE the real
            # (DMA-paced) projection matmuls arrive
            scr = consts.tile([128, QHALF], bf16, name="scr", tag="scr")
            nc.vector.memset(scr, 0.0)
            warm = psz.tile([128, QHALF], f32, name="warm", tag="z")
            for i in range(12):
                nc.tensor.matmul(warm[0:64, :], lhsT=ones_sb, rhs=scr,
                                 start=True, stop=True)

            # ---- qkv projections --------------------------------------------
            qT = [qkvp.tile([128, S], bf16, name=f"qT{m}", tag=f"qT{m}")
                  for m in range(MC)]
            kTt = [qkvp.tile([128, S], bf16, name=f"kT{m}", tag=f"kT{m}")
                   for m in range(MC)]
            v_sb = [qkvp.tile([128, OC], bf16, name=f"v{s}", tag=f"v{s}")
                    for s in range(KC)]

            # the scores ring: 3 slices of [128, 1024] fp32 in one tensor so
            # a single FD=2048 exp can span a slice pair
            ring = psr.tile([128, RING * S], f32, name="ring", tag="ring")

            def emit_qk_part(m, wname, sh, lead_in=False, ring_slot=None):
                """One q-half of one projection output chunk: 8 matmuls into a
                PSUM bank + DVE evacuation. Lead-in pieces use ring slices
                (scores haven't started); deferred pieces share the z bank."""
                dstT = qT if wname == "q" else kTt
                bias_sb = bqs_sb if wname == "q" else bks_sb
                if lead_in:
                    ps = ring[:, ring_slot * QHALF:(ring_slot + 1) * QHALF]
                else:
                    ps = psz.tile([128, QHALF], f32,
                                  name=f"ps{wname}{m}_{sh}", tag="z")
                for k in range(KC):
                    nc.tensor.matmul(
                        ps,
                        lhsT=w_sb[wname][k][:, m * 128:(m + 1) * 128],
                        rhs=x_sb[k][:, sh * QHALF:(sh + 1) * QHALF],
                        start=(k == 0), stop=(k == KC - 1),
                    )
                dst = dstT[m][:, sh * QHALF:(sh + 1) * QHALF]
                if apply_qkbias:
                    nc.scalar.activation(out=dst, in_=ps, func=AF.Identity,
                                         bias=bias_sb[:, m:m + 1], scale=1.0)
                else:
                    nc.vector.tensor_copy(dst, ps)

            def emit_v(s):
                ps = psc.tile([128, QHALF], f32, name=f"psv{s}", tag="ctx")
                for k in range(KC):
                    nc.tensor.matmul(
                        ps, lhsT=x_sb[k][:, s * 128:(s + 1) * 128],
                        rhs=w_sb["v"][k], start=(k == 0), stop=(k == KC - 1),
                    )
                nc.vector.tensor_copy(v_sb[s], ps)
                if apply_vbias:
                    nc.vector.tensor_add(v_sb[s], v_sb[s], bvb_sb)

            # qk0 needs only x and wq/wk: it fills the PE during the input
            # DMA window (using ring slices as scratch psum)
            for i, (wname, sh) in enumerate(
                    [("q", 0), ("q", 1), ("k", 0), ("k", 1)]):
                emit_qk_part(0, wname, sh, lead_in=True, ring_slot=i)

            # deferred projection work, spread across the attention loop so
            # the PE never idles long enough for HAM to re-throttle
            pend = {
                0: [("qk", 1, ("q", 0)), ("v", 0, None),
                    ("qk", 1, ("q", 1)), ("v", 1, None),
                    ("qk", 1, ("k", 0)), ("v", 2, None),
                    ("qk", 1, ("k", 1)), ("v", 3, None),
                    ("v", 4, None), ("v", 5, None),
                    ("v", 6, None), ("v", 7, None)],
                1: [("qk", 2, ("q", 0)), ("qk", 2, ("q", 1)),
                    ("qk", 2, ("k", 0)), ("qk", 2, ("k", 1))],
                2: [("qk", 3, ("q", 0)), ("qk", 3, ("q", 1)),
                    ("qk", 3, ("k", 0)), ("qk", 3, ("k", 1))],
                3: [],
            }

            def emit_pending(items, n):
                for _ in range(n):
                    if not items:
                        return
                    kind, idx, part = items.pop(0)
                    if kind == "qk":
                        emit_qk_part(idx, *part)
                    else:
                        emit_v(idx)

            # ---- attention, one head-pair at a time -------------------------
            # per (hp, c): 4 score matmuls -> 2 ring slices (h0 in the lower
            # one), 1 exp FD=2048 -> ex2 [128,2048] (h0|h1), 2 pt muls, and
            # (c odd) 1 pair add FD=2048. Z/ctx/recip/store for hp carried
            # into hp+1's phase in small chunks (as in the baseline).
            state = {}
            psx_tiles = {}
            rb_tiles = {}
            out_tiles = {}
            zq_tiles = {}
            ring_n = 0  # global ring slice counter

            def emit_ctx_chain(hp, qh, ps_x):
                pts = state[hp][0]
                for c in range(KC):
                    for half in range(2):
                        h = 2 * hp + half
                        nc.tensor.matmul(
                            ps_x[half * 64:(half + 1) * 64, :],
                            lhsT=v_sb[c][:, h * 64:(h + 1) * 64],
                            rhs=pts[c][:, half * S + qh * QHALF:
                                       half * S + qh * QHALF + QHALF],
                            start=(c == 0), stop=(c == KC - 1),
                            tile_position=(0, half * 64),
                            skip_group_check=True,
                        )

            def emit_z_chain(hp, qh):
                pairs = state[hp][1]
                if qh == 0:
                    rb_tiles[hp] = rbp.tile([128, S], f32, name=f"rb{hp}",
                                            tag="rb")
                zq = psz.tile([128, QHALF], f32, name=f"zq{hp}_{qh}", tag="z")
                zq_tiles[(hp, qh)] = zq
                # col-group alternating order so (half0, half1) pairs overlap
                for j in range(4):
                    for half in range(2):
                        nc.tensor.matmul(
                            zq[half * 64:(half + 1) * 64, :],
                            lhsT=ones_sb,
                            rhs=pairs[j][:, half * S + qh * QHALF:
                                         half * S + qh * QHALF + QHALF],
                            start=(j == 0), stop=(j == 3),
                            tile_position=(0, half * 64),
                            skip_group_check=True,
                        )

            def emit_recip(hp, qh):
                # 1/Z = exp(-ln(Z)); Ln and Exp live in one ACT table set.
                lnz = lnp.tile([128, QHALF], f32, name=f"lnz{hp}_{qh}",
                               tag="lnz")
                nc.scalar.activation(out=lnz, in_=zq_tiles[(hp, qh)],
                                     func=AF.Ln, bias=0.0, scale=1.0)
                nc.scalar.activation(
                    out=rb_tiles[hp][:, qh * QHALF:(qh + 1) * QHALF], in_=lnz,
                    func=AF.Exp, bias=0.0, scale=-1.0)

            def emit_ctx_qh0_a(hp):
                ps_x = psc.tile([128, QHALF], f32, name=f"px{hp}_0", tag="ctx")
                psx_tiles[hp] = ps_x
                pts = state[hp][0]
                for c in range(KC // 2):
                    for half in range(2):
                        h = 2 * hp + half
                        nc.tensor.matmul(
                            ps_x[half * 64:(half + 1) * 64, :],
                            lhsT=v_sb[c][:, h * 64:(h + 1) * 64],
                            rhs=pts[c][:, half * S:half * S + QHALF],
                            start=(c == 0), stop=False,
                            tile_position=(0, half * 64),
                            skip_group_check=True,
                        )

            def emit_ctx_qh0_b(hp):
                ps_x = psx_tiles[hp]
                pts = state[hp][0]
                for c in range(KC // 2, KC):
                    for half in range(2):
                        h = 2 * hp + half
                        nc.tensor.matmul(
                            ps_x[half * 64:(half + 1) * 64, :],
                            lhsT=v_sb[c][:, h * 64:(h + 1) * 64],
                            rhs=pts[c][:, half * S:half * S + QHALF],
                            start=False, stop=(c == KC - 1),
                            tile_position=(0, half * 64),
                            skip_group_check=True,
                        )
                outt = outp.tile([128, S], bf16, name=f"o{hp}", tag="o")
                out_tiles[hp] = outt
                nc.vector.tensor_mul(outt[:, 0:QHALF], ps_x,
                                     rb_tiles[hp][:, 0:QHALF])

            def emit_ctx_part2(hp):
                # the last pair's qh1 chain runs in the tail, when the ring
                # is free: borrow a ring slice so both chains overlap
                if hp == MC - 1:
                    ps_x = ring[:, 0:QHALF]
                else:
                    ps_x = psc.tile([128, QHALF], f32, name=f"px{hp}_1",
                                    tag="ctx")
                emit_ctx_chain(hp, 1, ps_x)
                outt = out_tiles[hp]
                nc.vector.tensor_mul(outt[:, QHALF:S], ps_x,
                                     rb_tiles[hp][:, QHALF:S])
                nc.sync.dma_start(out=outT[hp * 128:(hp + 1) * 128, :], in_=outt)

            carry = []
            for hp in range(MC):
                ex2s = {}
                pts = {}
                pairs = {}
                items = pend[hp]
                for c in range(KC):
                    # two ring slices for this (hp, c); h0 -> lower address
                    sa, sb = ring_n % RING, (ring_n + 1) % RING
                    ring_n += 2
                    lo, hi = min(sa, sb), max(sa, sb)
                    # 4 score matmuls, row-group interleaved for pairing
                    for qh in range(2):
                        for half, sl in ((0, lo), (1, hi)):
                            pr = half * 64
                            nc.tensor.matmul(
                                ring[:, sl * S + qh * QHALF:
                                     sl * S + qh * QHALF + QHALF],
                                lhsT=kTt[hp][pr:pr + 64, c * 128:(c + 1) * 128],
                                rhs=qT[hp][pr:pr + 64,
                                           qh * QHALF:(qh + 1) * QHALF],
                                start=True, stop=True,
                                tile_position=(pr, 0),
                            )
                    # one exp over both slices (h0 at lower address)
                    ex2 = expp.tile([128, 2 * S], bf16, name=f"e{hp}_{c}",
                                    tag="ex")
                    bias = am_sb[:, c:c + 1] if apply_am else 0.0
                    if hi == lo + 1:
                        nc.scalar.activation(
                            out=ex2, in_=ring[:, lo * S:(hi + 1) * S],
                            func=AF.Exp, bias=bias, scale=1.0)
                    else:
                        # ring wrap: two contiguous FD=1024 exps (a strided
                        # 3D ACT read miscompiles on hw)
                        nc.scalar.activation(
                            out=ex2[:, 0:S], in_=ring[:, lo * S:(lo + 1) * S],
                            func=AF.Exp, bias=bias, scale=1.0)
                        nc.scalar.activation(
                            out=ex2[:, S:2 * S], in_=ring[:, hi * S:(hi + 1) * S],
                            func=AF.Exp, bias=bias, scale=1.0)
                    ex2s[c] = ex2
                    pt2 = ptp.tile([128, 2 * S], bf16, name=f"p{hp}_{c}",
                                   tag="pt")
                    nc.vector.tensor_mul(pt2[:, 0:S], ex2[:, 0:S], lk_sb[c])
                    nc.vector.tensor_mul(pt2[:, S:2 * S], ex2[:, S:2 * S],
                                         lk_sb[c])
                    pts[c] = pt2
                    if c % 2 == 1:
                        par = parp.tile([128, 2 * S], bf16,
                                        name=f"par{hp}_{c // 2}", tag="par")
                        nc.vector.tensor_add(par, ex2s[c - 1], ex2)
                        pairs[c // 2] = par
                    # keep PE fed with projection matmuls for later pairs
                    if hp == 0:
                        emit_pending(items, 2 if c < 4 else 1)
                    elif c >= 4:
                        emit_pending(items, 1)
                    # one chunk of the previous pair's Z/ctx/store work per
                    # c-slot; the boundary slot c0 stays clear
                    if carry and c >= 1:
                        carry.pop(0)()

                state[hp] = (pts, pairs)
                carry = [
                    (lambda h=hp: emit_z_chain(h, 0)),
                    (lambda h=hp: emit_ctx_qh0_a(h)),
                    (lambda h=hp: (emit_recip(h, 0), emit_z_chain(h, 1))),
                    (lambda h=hp: (emit_recip(h, 1), emit_ctx_qh0_b(h))),
                    (lambda h=hp: emit_ctx_part2(h)),
                ]
            for f in carry:
                f()

    return nc


LAST_RESULT = None


def kernel(hidden_states, attention_mask, link_mask, Wq, bq, Wk, bk, Wv, bv):
    from concourse.bass_utils import run_bass_kernel_spmd

    hidden_states = np.asarray(hidden_states, np.float32)
    attention_mask = np.asarray(attention_mask, np.float32)
    link_mask = np.asarray(link_mask, np.float32)
    Wq, bq = np.asarray(Wq, np.float32), np.asarray(bq, np.float32)
    Wk, bk = np.asarray(Wk, np.float32), np.asarray(bk, np.float32)
    Wv, bv = np.asarray(Wv, np.float32), np.asarray(bv, np.float32)

    apply_qkbias = bool(np.any(bq)) or bool(np.any(bk))
    apply_am = bool(np.any(attention_mask))
    apply_vbias = bool(np.any(bv))
    nc = _build(apply_qkbias, apply_vbias, apply_am)

    in_maps = []
    for core in range(NCORES):
        b, hg = divmod(core, HG)
        sl = slice(hg * OC, (hg + 1) * OC)
        in_maps.append({
            "xT": np.ascontiguousarray(hidden_states[b].T).astype(BF16),
            "wq": np.ascontiguousarray(Wq[sl, :].T * 0.125).astype(BF16),
            "wk": np.ascontiguousarray(Wk[sl, :].T).astype(BF16),
            "wv": np.ascontiguousarray(Wv[sl, :].T).astype(BF16),
            "lkT": np.ascontiguousarray(link_mask[b, 0].T).astype(BF16),
            "am": np.ascontiguousarray(
                attention_mask[b, 0, 0].reshape(KC, 128).T).astype(np.float32),
            "bqs": np.ascontiguousarray(
                (bq[sl] / 8.0).reshape(MC, 128).T).astype(np.float32),
            "bks": np.ascontiguousarray(
                bk[sl].reshape(MC, 128).T).astype(np.float32),
            "bvb": bv[sl].reshape(1, OC).astype(BF16),
        })

    res = run_bass_kernel_spmd(nc, in_maps, core_ids=list(range(NCORES)))
    global LAST_RESULT
    LAST_RESULT = res

    out = np.empty((B, S, DM), np.float32)
    for core in range(NCORES):
        b, hg = divmod(core, HG)
        out[b, :, hg * OC:(hg + 1) * OC] = res.results[core]["outT"].T.astype(np.float32)
    return out


# revision 8
# speedup vs baseline: 1.0060x; 1.0060x over previous
"""Trainium2 Bass kernel for nn_LAtAttrRobertaSelfAttention.

ref:  q = split_heads(x @ Wq.T + bq); k, v likewise
      scores = q k^T / sqrt(D) + attention_mask          [B,H,S,S]
      probs  = softmax(scores, -1) * link_mask           (link broadcast over H)
      out    = merge_heads(probs @ v)                    [B,S,DM]

Sharding: 8 cores = 4 batches x 2 head-groups (8 heads each).

Per-core layout (all matmul operands bf16, fp32 accumulate):
  xT   [DM,S]   hidden_states[b].T       (host pre-transposed)
  wq/wk/wv [DM,OC] weight-slice.T for this head group (OC=512; q pre-scaled
      by 1/sqrt(D) on the host)
  qT,kT [OC,S]  = W.T-stationary matmuls; v [S,OC] = xT-stationary.
  sT[ki,qi] = kT-slice stationary @ qT-slice moving (softmax axis on
      partitions: probs are contraction-major for ctx, no transposes).

v2 over the serial baseline — exploit PE tile_position concurrency
(HW-measured: row-paired K=64 and col-paired M=64 matmuls with different
moving operands both run 2x):
  * scores PSUM is a manual 3-slice ring [128, 3*1024] (6 banks). The 4
    matmuls of a (c, head-pair) group write TWO slices with row-group
    interleaved order (h0q0, h1q0, h0q1, h1q1) -> 2 concurrent slots.
    h0 always lands in the lower-addressed slice so downstream tiles have
    a uniform (h0|h1) layout.
  * one ACT exp per slice-PAIR (FD=2048, ~12% cheaper per element than
    FD=1024 and frees two slices at once -- which is what makes the next
    4-matmul group pairable).
  * ctx and Z all-ones chains emit col-group alternating (tile_position
    (0,0)/(0,64)) -> 2x.
  * Z via DVE pair-sums of ex2 tiles (FD=2048 adds), denominator
    broadcast by all-ones matmuls, 1/Z = exp(-ln Z) on ACT (one table set).
  * deferred qk/v projection matmuls (serial K=128, no tiling possible)
    fill the PE between exp-paced score groups, as in the baseline.
  Output written as outT [OC,S] bf16; host transposes back.
"""

import functools

import numpy as np
import ml_dtypes

BF16 = ml_dtypes.bfloat16

B, S, DM, H = 4, 1024, 1024, 16
D = 64                # head dim
HG = 2                # head groups (tensor-parallel factor)
HL = H // HG          # 8 heads per core
OC = HL * D           # 512 output channels per core
NCORES = B * HG       # 8
KC = DM // 128        # 8 contraction chunks of 128
MC = OC // 128        # 4 o-chunks
QHALF = 512           # qi processed in halves (PSUM bank = 512 fp32)
RING = 6              # scores psum ring slices of [128, 512]


def _patch_tile_drain():
    """walrus in this container rejects instructions carrying more than one
    sync wait ("Too many sync wait commands"). Tile freely attaches several.
    Two patches: (1) split excess waits off every scheduled instruction onto
    single-wait NoOps committed just before it (same engine, so program order
    preserves the blocking semantics); (2) same treatment for the TileContext
    exit drain, which carries one wait per live proc."""
    import concourse.mybir as mybir
    import concourse.tile as ctile
    from concourse.vector_clock import ScopedClock

    MAXW = 1

    if not getattr(ctile.TileContext, "_ant_split_waits_patched", False):
        orig_commit = ctile.TileContext._commit_instruction

        def _commit_instruction(self, inst, lazy_reg_writes=True):
            if isinstance(inst, mybir.Instruction):
                si = inst.sync_info
                waits = list(si.on_wait) if si is not None and si.on_wait else []
                if len(waits) > MAXW:
                    for i in range(0, len(waits) - MAXW, MAXW):
                        nop = mybir.InstNoOp(
                            name=f"{inst.name}_w{i}",
                            engine=inst.engine,
                            sync_info=mybir.SyncInfo(
                                on_wait=waits[i:i + MAXW], on_update=[]),
                            bass_nofuse=True,
                        )
                        orig_commit(self, nop, lazy_reg_writes)
                    inst.sync_info = mybir.SyncInfo(
                        on_wait=waits[len(waits) - MAXW:],
                        on_update=(si.on_update or []),
                    )
            return orig_commit(self, inst, lazy_reg_writes)

        ctile.TileContext._commit_instruction = _commit_instruction
        ctile.TileContext._ant_split_waits_patched = True

    def _drain_and_barrier(self, tick_clock, wait_clock):
        nc = self.nc
        drain_inst = nc.sync.drain()
        wait_clock.add_sem_waits(
            drain_inst.ins, ScopedClock({None: tick_clock.global_clock})
        )
        si = drain_inst.ins.sync_info
        waits = list(si.on_wait or []) if si is not None else []
        if len(waits) > 1:
            drain_inst.ins.sync_info = mybir.SyncInfo(
                on_wait=[waits[0]], on_update=(si.on_update or [])
            )
            for w in waits[1:]:
                extra = nc.sync.drain()
                extra.ins.sync_info = mybir.SyncInfo(on_wait=[w], on_update=[])
        nc.all_engine_barrier()
        assert self.sems is not None
        popped = nc._tile_sem_poison_stack.pop()
        assert popped is self._sem_poison
        nc.clear_and_free_semaphores(list(self.sems.allocated().values()))
        nc.all_engine_barrier()

    ctile.TileContext._drain_and_barrier = _drain_and_barrier


@functools.lru_cache(maxsize=None)
def _build(apply_qkbias: bool, apply_vbias: bool, apply_am: bool):
    import concourse.bass as bass
    import concourse.mybir as mybir
    import concourse.tile as tile

    _patch_tile_drain()

    f32 = mybir.dt.float32
    bf16 = mybir.dt.bfloat16
    AF = mybir.ActivationFunctionType

    nc = bass.Bass("TRN2")
    xT = nc.dram_tensor("xT", [DM, S], bf16, kind="ExternalInput")
    wq = nc.dram_tensor("wq", [DM, OC], bf16, kind="ExternalInput")
    wk = nc.dram_tensor("wk", [DM, OC], bf16, kind="ExternalInput")
    wv = nc.dram_tensor("wv", [DM, OC], bf16, kind="ExternalInput")
    lkT = nc.dram_tensor("lkT", [S, S], bf16, kind="ExternalInput")
    am = nc.dram_tensor("am", [128, KC], f32, kind="ExternalInput")
    bqs = nc.dram_tensor("bqs", [128, MC], f32, kind="ExternalInput")
    bks = nc.dram_tensor("bks", [128, MC], f32, kind="ExternalInput")
    bvb = nc.dram_tensor("bvb", [1, OC], bf16, kind="ExternalInput")
    outT = nc.dram_tensor("outT", [OC, S], bf16, kind="ExternalOutput")

    with tile.TileContext(nc) as tc:
        with (
            tc.tile_pool(name="consts", bufs=1) as consts,
            tc.tile_pool(name="qkv", bufs=1) as qkvp,
            tc.tile_pool(name="expp", bufs=4) as expp,
            tc.tile_pool(name="ptp", bufs=15) as ptp,
            tc.tile_pool(name="parp", bufs=6) as parp,
            tc.tile_pool(name="rbp", bufs=2) as rbp,
            tc.tile_pool(name="lnp", bufs=2) as lnp,
            tc.tile_pool(name="outp", bufs=2) as outp,
            tc.tile_pool(name="psr", bufs=1, space="PSUM") as psr,
            tc.tile_pool(name="psc", bufs=1, space="PSUM") as psc,
            tc.tile_pool(name="psz", bufs=1, space="PSUM") as psz,
        ):
            # ---- constant loads (interleaved so qk0 matmuls start ASAP) ------
            x_sb = [consts.tile([128, S], bf16, name=f"x{k}", tag=f"x{k}")
                    for k in range(KC)]
            w_sb = {wname: [consts.tile([128, OC], bf16, name=f"w{wname}{k}",
                                        tag=f"w{wname}{k}")
                            for k in range(KC)]
                    for wname in ("q", "k", "v")}
            lk_sb = [consts.tile([128, S], bf16, name=f"lk{c}", tag=f"lk{c}")
                     for c in range(KC)]
            # tiny per-partition constants FIRST — the exp stream's bias tile
            # (am_sb) must not queue behind 7MB of bulk input
            am_sb = consts.tile([128, KC], f32, name="am_sb", tag="am_sb")
            nc.sync.dma_start(out=am_sb, in_=am[:, :])
            bqs_sb = consts.tile([128, MC], f32, name="bqs_sb", tag="bqs_sb")
            nc.sync.dma_start(out=bqs_sb, in_=bqs[:, :])
            bks_sb = consts.tile([128, MC], f32, name="bks_sb", tag="bks_sb")
            nc.sync.dma_start(out=bks_sb, in_=bks[:, :])
            if apply_vbias:
                bvb_sb = consts.tile([128, OC], bf16, name="bvb_sb", tag="bvb_sb")
                nc.sync.dma_start(out=bvb_sb,
                                  in_=bvb[0:1, :].partition_broadcast(128))

            # input DMA is HBM-bandwidth-bound on one queue; order chunks by
            # when the compute needs them (x+wq first for qk0/qk1)
            for k in range(KC):
                nc.sync.dma_start(out=x_sb[k], in_=xT[k * 128:(k + 1) * 128, :])
                nc.sync.dma_start(out=w_sb["q"][k],
                                  in_=wq[k * 128:(k + 1) * 128, :])
            for k in range(KC):
                nc.sync.dma_start(out=w_sb["k"][k],
                                  in_=wk[k * 128:(k + 1) * 128, :])
            for k in range(KC):
                nc.sync.dma_start(out=w_sb["v"][k],
                                  in_=wv[k * 128:(k + 1) * 128, :])
            for c in range(KC):
                nc.sync.dma_start(out=lk_sb[c],
                                  in_=lkT[c * 128:(c + 1) * 128, :])
            ones_sb = consts.tile([128, 64], bf16, name="ones_sb", tag="ones_sb")
            nc.vector.memset(ones_sb, 1.0)
            # dummy matmuls on scratch data while the input DMA streams: the
            # PE's HAM clock-gate needs ~3.4us of sustained activity to lift
            # the array from 1.2GHz to 2.4GHz, so warm it up BEFORE the real
            # (DMA-paced) projection matmuls arrive
            scr = consts.tile([128, QHALF], bf16, name="scr", tag="scr")
            nc.vector.memset(scr, 0.0)
            warm = psz.tile([128, QHALF], f32, name="warm", tag="z")
            for i in range(12):
                nc.tensor.matmul(warm[0:64, :], lhsT=ones_sb, rhs=scr,
                                 start=True, stop=True)

            # ---- qkv projections --------------------------------------------
            qT = [qkvp.tile([128, S], bf16, name=f"qT{m}", tag=f"qT{m}")
                  for m in range(MC)]
            kTt = [qkvp.tile([128, S], bf16, name=f"kT{m}", tag=f"kT{m}")
                   for m in range(MC)]
            v_sb = [qkvp.tile([128, OC], bf16, name=f"v{s}", tag=f"v{s}")
                    for s in range(KC)]

            # the scores ring: 3 slices of [128, 1024] fp32 in one tensor so
            # a single FD=2048 exp can span a slice pair
            ring = psr.tile([128, RING * QHALF], f32, name="ring", tag="ring")

            def emit_qk_part(m, wname, sh, lead_in=False, ring_slot=None,
                             bank=None):
                """One q-half of one projection output chunk: 8 matmuls into a
                PSUM bank + DVE evacuation. Lead-in pieces use ring slices
                (scores haven't started); deferred pieces share the z bank."""
                dstT = qT if wname == "q" else kTt
                bias_sb = bqs_sb if wname == "q" else bks_sb
                if lead_in:
                    ps = ring[:, ring_slot * QHALF:(ring_slot + 1) * QHALF]
                elif bank is psc:
                    ps = psc.tile([128, QHALF], f32,
                                  name=f"ps{wname}{m}_{sh}", tag="ctx")
                else:
                    ps = psz.tile([128, QHALF], f32,
                                  name=f"ps{wname}{m}_{sh}", tag="z")
                for k in range(KC):
                    nc.tensor.matmul(
                        ps,
                        lhsT=w_sb[wname][k][:, m * 128:(m + 1) * 128],
                        rhs=x_sb[k][:, sh * QHALF:(sh + 1) * QHALF],
                        start=(k == 0), stop=(k == KC - 1),
                    )
                dst = dstT[m][:, sh * QHALF:(sh + 1) * QHALF]
                if apply_qkbias:
                    nc.scalar.activation(out=dst, in_=ps, func=AF.Identity,
                                         bias=bias_sb[:, m:m + 1], scale=1.0)
                else:
                    nc.vector.tensor_copy(dst, ps)

            def emit_v(s, bank=None):
                if bank is psz:
                    ps = psz.tile([128, QHALF], f32, name=f"psv{s}", tag="z")
                else:
                    ps = psc.tile([128, QHALF], f32, name=f"psv{s}", tag="ctx")
                for k in range(KC):
                    nc.tensor.matmul(
                        ps, lhsT=x_sb[k][:, s * 128:(s + 1) * 128],
                        rhs=w_sb["v"][k], start=(k == 0), stop=(k == KC - 1),
                    )
                nc.vector.tensor_copy(v_sb[s], ps)
                if apply_vbias:
                    nc.vector.tensor_add(v_sb[s], v_sb[s], bvb_sb)

            # qk0 needs only x and wq/wk: it fills the PE during the input
            # DMA window (using ring slices as scratch psum)
            for i, (wname, sh) in enumerate(
                    [("q", 0), ("q", 1), ("k", 0), ("k", 1)]):
                emit_qk_part(0, wname, sh, lead_in=True, ring_slot=i)

            # deferred projection work, spread across the attention loop so
            # the PE never idles long enough for HAM to re-throttle
            # deferred-projection placement: per (hp, c-slot) piece lists,
            # bank-assigned around the carried Z/ctx chains' psum residency
            # (psz held c1-c4 by z chains; psc held c2-c7 by ctx chains).
            C, Z = "psc", "psz"
            sched = {
                0: [[("v", 0, C), ("qk", 1, ("q", 0), Z)],
                    [("v", 1, C), ("qk", 1, ("q", 1), Z)],
                    [("v", 2, C)], [("v", 3, C)],
                    [("qk", 1, ("k", 0), Z)], [("v", 4, C)],
                    [("qk", 1, ("k", 1), Z)], [("v", 5, C)]],
                1: [[("v", 6, C)], [("v", 7, C)], [], [],
                    [("qk", 2, ("q", 0), Z)], [("qk", 2, ("q", 1), Z)],
                    [("qk", 2, ("k", 0), Z)], [("qk", 2, ("k", 1), Z)]],
                2: [[], [], [], [],
                    [("qk", 3, ("q", 0), Z)], [("qk", 3, ("q", 1), Z)],
                    [("qk", 3, ("k", 0), Z)], [("qk", 3, ("k", 1), Z)]],
                3: [[], [], [], [], [], [], [], []],
            }

            def emit_pieces(pieces):
                for p in pieces:
                    bank = psc if p[-1] == C else psz
                    if p[0] == "qk":
                        emit_qk_part(p[1], *p[2], bank=bank)
                    else:
                        emit_v(p[1], bank=bank)

            # ---- attention, one head-pair at a time -------------------------
            # per (hp, c): 4 score matmuls -> 2 ring slices (h0 in the lower
            # one), 1 exp FD=2048 -> ex2 [128,2048] (h0|h1), 2 pt muls, and
            # (c odd) 1 pair add FD=2048. Z/ctx/recip/store for hp carried
            # into hp+1's phase in small chunks (as in the baseline).
            state = {}
            psx_tiles = {}
            rb_tiles = {}
            out_tiles = {}
            zq_tiles = {}
            ring_n = 0  # global ring slice counter

            def emit_ctx_chain(hp, qh, ps_x):
                pts = state[hp][0]
                for c in range(KC):
                    for half in range(2):
                        h = 2 * hp + half
                        nc.tensor.matmul(
                            ps_x[half * 64:(half + 1) * 64, :],
                            lhsT=v_sb[c][:, h * 64:(h + 1) * 64],
                            rhs=pts[c][:, qh * S + half * QHALF:
                                       qh * S + half * QHALF + QHALF],
                            start=(c == 0), stop=(c == KC - 1),
                            tile_position=(0, half * 64),
                            skip_group_check=True,
                        )

            def emit_z_chain(hp, qh):
                pairs = state[hp][1]
                if qh == 0:
                    rb_tiles[hp] = rbp.tile([128, S], f32, name=f"rb{hp}",
                                            tag="rb")
                zq = psz.tile([128, QHALF], f32, name=f"zq{hp}_{qh}", tag="z")
                zq_tiles[(hp, qh)] = zq
                # col-group alternating order so (half0, half1) pairs overlap
                for j in range(4):
                    for half in range(2):
                        nc.tensor.matmul(
                            zq[half * 64:(half + 1) * 64, :],
                            lhsT=ones_sb,
                            rhs=pairs[j][:, qh * S + half * QHALF:
                                         qh * S + half * QHALF + QHALF],
                            start=(j == 0), stop=(j == 3),
                            tile_position=(0, half * 64),
                            skip_group_check=True,
                        )

            def emit_recip(hp, qh):
                # 1/Z = exp(-ln(Z)); Ln and Exp live in one ACT table set.
                lnz = lnp.tile([128, QHALF], f32, name=f"lnz{hp}_{qh}",
                               tag="lnz")
                nc.scalar.activation(out=lnz, in_=zq_tiles[(hp, qh)],
                                     func=AF.Ln, bias=0.0, scale=1.0)
                nc.scalar.activation(
                    out=rb_tiles[hp][:, qh * QHALF:(qh + 1) * QHALF], in_=lnz,
                    func=AF.Exp, bias=0.0, scale=-1.0)

            def emit_ctx_qh0_a(hp):
                ps_x = psc.tile([128, QHALF], f32, name=f"px{hp}_0", tag="ctx")
                psx_tiles[hp] = ps_x
                pts = state[hp][0]
                for c in range(KC // 2):
                    for half in range(2):
                        h = 2 * hp + half
                        nc.tensor.matmul(
                            ps_x[half * 64:(half + 1) * 64, :],
                            lhsT=v_sb[c][:, h * 64:(h + 1) * 64],
                            rhs=pts[c][:, half * QHALF:(half + 1) * QHALF],
                            start=(c == 0), stop=False,
                            tile_position=(0, half * 64),
                            skip_group_check=True,
                        )

            def emit_ctx_qh0_b(hp):
                ps_x = psx_tiles[hp]
                pts = state[hp][0]
                for c in range(KC // 2, KC):
                    for half in range(2):
                        h = 2 * hp + half
                        nc.tensor.matmul(
                            ps_x[half * 64:(half + 1) * 64, :],
                            lhsT=v_sb[c][:, h * 64:(h + 1) * 64],
                            rhs=pts[c][:, half * QHALF:(half + 1) * QHALF],
                            start=False, stop=(c == KC - 1),
                            tile_position=(0, half * 64),
                            skip_group_check=True,
                        )
                outt = outp.tile([128, S], bf16, name=f"o{hp}", tag="o")
                out_tiles[hp] = outt
                nc.vector.tensor_mul(outt[:, 0:QHALF], ps_x,
                                     rb_tiles[hp][:, 0:QHALF])

            def emit_ctx_part2(hp):
                # the last pair's qh1 chain runs in the tail, when the ring
                # is free: borrow a ring slice so both chains overlap
                if hp == MC - 1:
                    ps_x = ring[:, 0:QHALF]  # ring free in the tail
                else:
                    ps_x = psc.tile([128, QHALF], f32, name=f"px{hp}_1",
                                    tag="ctx")
                emit_ctx_chain(hp, 1, ps_x)
                outt = out_tiles[hp]
                nc.vector.tensor_mul(outt[:, QHALF:S], ps_x,
                                     rb_tiles[hp][:, QHALF:S])
                nc.sync.dma_start(out=outT[hp * 128:(hp + 1) * 128, :], in_=outt)

            carry = []
            for hp in range(MC):
                ex2s = {}
                pts = {}
                pairs = {}
                for c in range(KC):
                    # window of 4 ring slices [w0..w3], advance 4 mod 6.
                    # w0,w1 were freed 2 exps ago; w2,w3 by the PREVIOUS exp.
                    # qh-major head assignment puts the gated pair on q1 of
                    # both heads so both MM pairs are row-group alternating:
                    #   h0q0->w0, h1q0->w1 (free, run under exp g-1)
                    #   h0q1->w2, h1q1->w3 (gated, run right after it)
                    g = ring_n
                    ring_n += 1
                    w = [(4 * g + i) % RING for i in range(4)]
                    for j, (half, qh) in enumerate(
                            ((0, 0), (1, 0), (0, 1), (1, 1))):
                        pr = half * 64
                        nc.tensor.matmul(
                            ring[:, w[j] * QHALF:(w[j] + 1) * QHALF],
                            lhsT=kTt[hp][pr:pr + 64, c * 128:(c + 1) * 128],
                            rhs=qT[hp][pr:pr + 64,
                                       qh * QHALF:(qh + 1) * QHALF],
                            start=True, stop=True,
                            tile_position=(pr, 0),
                        )
                    # exp over the window -> ex2 [q0:(h0|h1) | q1:(h0|h1)].
                    # windows are contiguous except the g%3==1 wrap, which
                    # splits into two contiguous FD=1024 exps (strided 3D
                    # ACT reads miscompile on hw).
                    ex2 = expp.tile([128, 2 * S], bf16, name=f"e{hp}_{c}",
                                    tag="ex")
                    bias = am_sb[:, c:c + 1] if apply_am else 0.0
                    if w[0] + 3 == w[3]:
                        nc.scalar.activation(
                            out=ex2,
                            in_=ring[:, w[0] * QHALF:(w[3] + 1) * QHALF],
                            func=AF.Exp, bias=bias, scale=1.0)
                    else:
                        nc.scalar.activation(
                            out=ex2[:, 0:S],
                            in_=ring[:, w[0] * QHALF:(w[1] + 1) * QHALF],
                            func=AF.Exp, bias=bias, scale=1.0)
                        nc.scalar.activation(
                            out=ex2[:, S:2 * S],
                            in_=ring[:, w[2] * QHALF:(w[3] + 1) * QHALF],
                            func=AF.Exp, bias=bias, scale=1.0)
                    ex2s[c] = ex2
                    pt2 = ptp.tile([128, 2 * S], bf16, name=f"p{hp}_{c}",
                                   tag="pt")
                    for j, (half, qh) in enumerate(
                            ((0, 0), (1, 0), (0, 1), (1, 1))):
                        nc.vector.tensor_mul(
                            pt2[:, j * QHALF:(j + 1) * QHALF],
                            ex2[:, j * QHALF:(j + 1) * QHALF],
                            lk_sb[c][:, qh * QHALF:(qh + 1) * QHALF])
                    pts[c] = pt2
                    if c % 2 == 1:
                        par = parp.tile([128, 2 * S], bf16,
                                        name=f"par{hp}_{c // 2}", tag="par")
                        nc.vector.tensor_add(par, ex2s[c - 1], ex2)
                        pairs[c // 2] = par
                    # one chunk of the previous pair's Z/ctx/store work per
                    # c-slot; the boundary slot c0 stays clear
                    if carry and c >= 1:
                        carry.pop(0)()
                    # keep PE fed with projection matmuls for later pairs
                    emit_pieces(sched[hp][c])

                state[hp] = (pts, pairs)
                carry = [
                    (lambda h=hp: emit_z_chain(h, 0)),
                    (lambda h=hp: emit_ctx_qh0_a(h)),
                    (lambda h=hp: (emit_recip(h, 0), emit_z_chain(h, 1))),
                    (lambda h=hp: emit_recip(h, 1)),
                    (lambda h=hp: emit_ctx_qh0_b(h)),
                    (lambda h=hp: emit_ctx_part2(h)),
                ]
            for f in carry:
                f()

    return nc


LAST_RESULT = None


def kernel(hidden_states, attention_mask, link_mask, Wq, bq, Wk, bk, Wv, bv):
    from concourse.bass_utils import run_bass_kernel_spmd

    hidden_states = np.asarray(hidden_states, np.float32)
    attention_mask = np.asarray(attention_mask, np.float32)
    link_mask = np.asarray(link_mask, np.float32)
    Wq, bq = np.asarray(Wq, np.float32), np.asarray(bq, np.float32)
    Wk, bk = np.asarray(Wk, np.float32), np.asarray(bk, np.float32)
    Wv, bv = np.asarray(Wv, np.float32), np.asarray(bv, np.float32)

    apply_qkbias = bool(np.any(bq)) or bool(np.any(bk))
    apply_am = bool(np.any(attention_mask))
    apply_vbias = bool(np.any(bv))
    nc = _build(apply_qkbias, apply_vbias, apply_am)

    in_maps = []
    for core in range(NCORES):
        b, hg = divmod(core, HG)
        sl = slice(hg * OC, (hg + 1) * OC)
        in_maps.append({
            "xT": np.ascontiguousarray(hidden_states[b].T).astype(BF16),
            "wq": np.ascontiguousarray(Wq[sl, :].T * 0.125).astype(BF16),
            "wk": np.ascontiguousarray(Wk[sl, :].T).astype(BF16),
            "wv": np.ascontiguousarray(Wv[sl, :].T).astype(BF16),
            "lkT": np.ascontiguousarray(link_mask[b, 0].T).astype(BF16),
            "am": np.ascontiguousarray(
                attention_mask[b, 0, 0].reshape(KC, 128).T).astype(np.float32),
            "bqs": np.ascontiguousarray(
                (bq[sl] / 8.0).reshape(MC, 128).T).astype(np.float32),
            "bks": np.ascontiguousarray(
                bk[sl].reshape(MC, 128).T).astype(np.float32),
            "bvb": bv[sl].reshape(1, OC).astype(BF16),
        })

    res = run_bass_kernel_spmd(nc, in_maps, core_ids=list(range(NCORES)))
    global LAST_RESULT
    LAST_RESULT = res

    out = np.empty((B, S, DM), np.float32)
    for core in range(NCORES):
        b, hg = divmod(core, HG)
        out[b, :, hg * OC:(hg + 1) * OC] = res.results[core]["outT"].T.astype(np.float32)
    return out
